# revision 7
# baseline (speedup 1.0000x reference)
"""Block-diagonal grouped GEMM (BlockDense) for Trainium2, 8 NeuronCores.

Problem: x:(8192, 16384) f32, W:(1024, 16, 16) f32
         out[b, g*16+h] = relu(sum_w x[b, g*16+w] * W[g, w, h])

Strategy:
  - Data-parallel shard of the batch dim across 8 cores (1024 rows each).
  - Host relayouts each x shard so features sit on SBUF partitions
    (the PE contracts along partitions); 8 groups are packed into one
    128x128 block-diagonal weight supergroup so the full PE array is used.
  - Per core: for each of 16 column blocks (1024 cols = 8 supergroups):
    DMA x-block + W-block, 64 fp32 matmuls (stationary = xT tile),
    relu PSUM->SBUF on alternating Scalar/Vector engines, DMA out.
"""

import numpy as np

import concourse.bass as bass
import concourse.mybir as mybir
import concourse.tile as tile
from concourse import bacc, bass_utils

# Problem constants (hardcoded per contract; kernel.py must be self-contained)
G, W_SZ, H = 1024, 16, 16
B = 8192
F = G * W_SZ  # 16384 input features = output features (H == W_SZ)
N_CORES = 8
B_LOC = B // N_CORES  # 1024 batch rows per core

P = 128          # partitions
GROUPS_PER_SG = 128 // W_SZ   # 8 groups per 128x128 supergroup
N_SG = G // GROUPS_PER_SG     # 128 supergroups
SG_PER_BLK = 8                # supergroups per column block
N_BLK = N_SG // SG_PER_BLK    # 16 column blocks of 1024 columns
BLK_COLS = SG_PER_BLK * P     # 1024
BT = B_LOC // P               # 8 batch tiles per core

_cached = {}


def _build_program():
    """Build the (single-core SPMD) bass program once per process."""
    if "nc" in _cached:
        return _cached["nc"]

    f32 = mybir.dt.float32
    nc = bacc.Bacc("TRN2", debug=False, target_bir_lowering=False)

    xt_d = nc.dram_tensor("xt", (N_BLK, P, SG_PER_BLK * B_LOC), f32,
                          kind="ExternalInput")
    # compact weights: [jj, w, sg, h] (1 MB)
    wc_d = nc.dram_tensor("wc", (GROUPS_PER_SG, W_SZ, N_SG, H), f32,
                          kind="ExternalInput")
    out_d = nc.dram_tensor("out", (B_LOC, F), f32, kind="ExternalOutput")

    xt_ap = xt_d.ap()
    wc_ap = wc_d.ap()
    out_ap = out_d.ap()

    relu = mybir.ActivationFunctionType.Relu

    with tile.TileContext(nc) as tc:
        with (
            tc.tile_pool(name="wpool", bufs=1) as wpool,
            tc.tile_pool(name="xpool", bufs=2) as xpool,
            tc.tile_pool(name="opool", bufs=6) as opool,
            tc.tile_pool(name="pspool", bufs=8, space=bass.MemorySpace.PSUM) as pspool,
        ):
            # Build the resident block-diagonal weight tile once:
            # wt_all[i, sg*128 + o] = W[8*sg+jj, w, h] for i=16jj+w, o=16jj+h
            wt_all = wpool.tile([P, N_SG * P], f32)
            third = (N_SG * P) // 4
            nc.vector.memset(wt_all[:, 0:third], 0.0)
            nc.scalar.memzero(wt_all[:, third:2 * third])
            nc.gpsimd.memset(wt_all[:, 2 * third:3 * third], 0.0)
            nc.vector.memset(wt_all[:, 3 * third:], 0.0)
            wt_view = wt_all[:].rearrange("p (sg o) -> p sg o", o=P)
            for jj in range(GROUPS_PER_SG):
                nc.sync.dma_start(
                    wt_view[16 * jj:16 * jj + 16, :, 16 * jj:16 * jj + 16],
                    wc_ap[jj],
                )

            for blk in range(N_BLK):
                # x block: [p, j*1024 + b] for 8 supergroups x full local batch
                xt_t = xpool.tile([P, SG_PER_BLK * B_LOC], f32)
                nc.sync.dma_start(xt_t[:], xt_ap[blk])

                for bt in range(BT):
                    ot = opool.tile([P, BLK_COLS], f32)
                    for half in range(2):
                        ps = pspool.tile([P, 512], f32)
                        for q in range(4):
                            j = half * 4 + q
                            sg = blk * SG_PER_BLK + j
                            nc.tensor.matmul(
                                ps[:, q * P:(q + 1) * P],
                                xt_t[:, j * B_LOC + bt * P: j * B_LOC + bt * P + P],
                                wt_all[:, sg * P:(sg + 1) * P],
                                start=True, stop=True,
                            )
                        dst = ot[:, half * 512:(half + 1) * 512]
                        if (bt * 2 + half) % 2 == 0:
                            nc.scalar.activation(dst, ps[:], relu)
                        else:
                            nc.vector.tensor_scalar_max(dst, ps[:], 0.0)
                    nc.sync.dma_start(
                        out_ap[bt * P:(bt + 1) * P,
                               blk * BLK_COLS:(blk + 1) * BLK_COLS],
                        ot[:],
                    )

    nc.compile()
    _cached["nc"] = nc
    return nc


def _prep_w(W: np.ndarray) -> np.ndarray:
    """Compact weights reordered to [jj, w, sg, h] for on-chip expansion."""
    Wr = np.ascontiguousarray(W, dtype=np.float32).reshape(
        N_SG, GROUPS_PER_SG, W_SZ, H)
    return np.ascontiguousarray(Wr.transpose(1, 2, 0, 3))


def _prep_x_shard(xs: np.ndarray) -> np.ndarray:
    """Relayout one (1024, 16384) shard to (16, 128, 8*1024).

    xt[blk, p, j*1024 + b] = xs[b, blk*1024 + j*128 + p]
    """
    x4 = xs.reshape(B_LOC, N_BLK, SG_PER_BLK, P)          # b, blk, j, p
    xt = np.ascontiguousarray(x4.transpose(1, 3, 2, 0))    # blk, p, j, b
    return xt.reshape(N_BLK, P, SG_PER_BLK * B_LOC)


# Debug/benchmark knobs (used by test.py only; harness leaves defaults)
TRACE = False
TRACE_CORES = None  # e.g. [0] or list(range(8))
LAST_RESULTS = None


def kernel(x: np.ndarray, W: np.ndarray) -> np.ndarray:
    global LAST_RESULTS
    assert x.shape == (B, F) and W.shape == (G, W_SZ, H)
    x = np.ascontiguousarray(x, dtype=np.float32)

    wc = _prep_w(W)
    in_maps = []
    for s in range(N_CORES):
        xs = x[s * B_LOC:(s + 1) * B_LOC]
        in_maps.append({"xt": _prep_x_shard(xs), "wc": wc})

    nc = _build_program()
    kwargs = {}
    if TRACE:
        kwargs = {"trace": True, "trace_cores": TRACE_CORES}
    res = bass_utils.run_bass_kernel_spmd(nc, in_maps,
                                          core_ids=list(range(N_CORES)),
                                          **kwargs)
    LAST_RESULTS = res
    out = np.concatenate([r["out"] for r in res.results], axis=0)
    return out
